# revision 18
# baseline (speedup 1.0000x reference)
"""Trainium2 Bass kernel for CrossAttention (LN -> QKV proj -> MHA -> out proj).

Sharding: data-parallel over (batch, query-half): 8 shards for B=4.
Each core gets a [1024, 1024] query-token slice and the full [2048, 768]
context for its batch, and produces a [1024, 1024] output slice.

v3 design notes (ACT-engine-centric, DMA-dispatch-lean):
  - The softmax exp (33.5M elems/core on the 128-lane 1.2GHz ACT engine,
    ~273us) is the true roofline; ACT runs (almost) nothing but exp.
  - LN's rsqrt runs on ACT as Exp(-0.5*Ln(var+eps)): Ln and Exp share one
    activation table set, so the kernel never swaps ACT tables (the
    baseline's Sqrt/Exp alternation costs ~2.7us per swap).
  - LN applies gamma/beta via two fused scalar_tensor_tensor passes:
    (x-mu)*gamma then *rsqrt+beta.  K's projection bias is dropped
    entirely (constant over the softmax axis); V's bias is added at the
    V-psum drain; Q's bias at the Q-psum drain (bq loaded directly in
    column layout - no matvecs, no DRAM bounces).
  - HWDGE dispatch (~0.6us/DMA) is a shared serial resource: token tiles
    and weights load in [128, 2, C] pairs, and each LN tile transposes
    with ONE xbar-transpose DMA into a [128, KC, tok] chunk tile
    (24 transposes total instead of 160).
  - Attention software-pipelines with a one-kt lag (attended(kt-1)
    emitted after scores(kt)) so the in-order PE queue never stalls on
    exp; scores for the even/odd head go q2-outer/par-inner so the two
    64-row matmuls land on disjoint PE row groups and can overlap.
  - Softmax denominators come from a ones-column appended to V (row 64
    of the attended psum); normalization: DVE reciprocal + DRAM-bounce
    partition broadcast.
"""

import numpy as np

import concourse.bass as bass
import concourse.tile as tile
from concourse import mybir
from concourse.bass_utils import run_bass_kernel_spmd

F32 = mybir.dt.float32
BF16 = mybir.dt.bfloat16
AF = mybir.ActivationFunctionType
OP = mybir.AluOpType

B, NQ_FULL, NK, CQ, CK, H, D = 4, 2048, 2048, 1024, 768, 16, 64
NQ = 1024            # per-core query tokens
N_CORES = 8
EPS = 1e-5
SM_SCALE = 1.0 / np.sqrt(D)  # 0.125

KC_Q = CQ // 128     # 8  contraction chunks for CQ
KC_C = CK // 128     # 6  contraction chunks for CK
NQT = NQ // 128      # 8  query token tiles
NKT = NK // 128      # 16 context token tiles
QC = 512             # 512-col psum tiles (fp32 bank limit)
NQ2 = NQ // QC       # 2
NG = NKT // 4        # 4  context 512-token groups


def _split_excess_waits(nc, max_waits=1):
    """walrus in this container accepts at most one sync wait per
    instruction; Tile's kernel-tail drain carries several.  Hoist excess
    waits onto single-wait NOPs that precede the instruction on the same
    engine (absolute sem waits commute, so this is semantics-preserving)."""
    for fn in nc.m.functions:
        for blk in fn.blocks:
            out = []
            dirty = False
            for inst in list(blk.instructions):
                si = inst.sync_info
                if si is not None and len(si.on_wait) > max_waits:
                    waits = list(si.on_wait)
                    for k, w in enumerate(waits[:-max_waits]):
                        nop = mybir.InstNoOp(
                            name=f"wsplit-{inst.name}-{k}", ins=[], outs=[])
                        nop.engine = inst.engine
                        nop.sync_info = mybir.SyncInfo(on_wait=[w], on_update=[])
                        out.append(nop)
                    inst.sync_info = mybir.SyncInfo(
                        on_wait=waits[-max_waits:], on_update=list(si.on_update))
                    dirty = True
                out.append(inst)
            if dirty:
                blk.instructions = out


def _bcast_ap(handle, n_parts, n_free):
    """DRAM [n_free] vector replicated across n_parts partitions."""
    return bass.AP(tensor=handle.ap().tensor, offset=0,
                   ap=[[0, n_parts], [1, n_free]])


def _cols_ap(handle, n_chunks):
    """DRAM [n_chunks*128] vector as [128, n_chunks] (partition-major)."""
    return bass.AP(tensor=handle.ap().tensor, offset=0,
                   ap=[[1, 128], [128, n_chunks]])


def _emit(tc, t, out, stages=("proj", "attn", "out")):
    from contextlib import ExitStack
    nc = tc.nc

    es = ExitStack()
    persist = es.enter_context(tc.tile_pool(name="persist", bufs=1))

    qTc = [persist.tile([128, NQ], BF16, tag=f"qT{oc}", name=f"qT{oc}")
           for oc in range(KC_Q)]
    kTc = [[persist.tile([128, QC], BF16, tag=f"kT{oc}_{t4}",
                         name=f"kT{oc}_{t4}") for t4 in range(NG)]
           for oc in range(KC_Q)]
    # V with a ones column appended per head: the attended-value matmul
    # then also emits the softmax denominator (row 64 of its psum)
    v_g = [persist.tile([128, 4, H, D + 1], BF16, tag=f"v{g}", name=f"v{g}")
           for g in range(NG)]
    bq_cols = persist.tile([128, KC_Q], F32, tag="bqc")
    eps_t = persist.tile([128, 1], F32, tag="eps")
    nc.vector.memset(eps_t[:, :], EPS)

    late = es.enter_context(tc.tile_pool(name="late", bufs=1))
    attT = late.tile([128, KC_Q, NQ], BF16, name="attT")

    scr = es.enter_context(tc.tile_pool(name="scr", bufs=4, space="DRAM"))

    # ---------------- phase 1: loads + LN + projections --------------
    proj_es = ExitStack()
    pps = proj_es.enter_context(tc.tile_pool(name="pps", bufs=2, space="PSUM"))
    lnw = proj_es.enter_context(tc.tile_pool(name="lnw", bufs=1))
    wfp = proj_es.enter_context(tc.tile_pool(name="wfp", bufs=2))
    xfp = proj_es.enter_context(tc.tile_pool(name="xfp", bufs=2))
    stp = proj_es.enter_context(tc.tile_pool(name="stp", bufs=2))
    bfp = proj_es.enter_context(tc.tile_pool(name="bfp", bufs=2))
    xTp = proj_es.enter_context(tc.tile_pool(name="xTp", bufs=2))

    # gamma/beta broadcast rows (fp32), direct bias column loads
    gqb = lnw.tile([128, CQ], F32, tag="gqb")
    bqb = lnw.tile([128, CQ], F32, tag="bqb")
    gcb = lnw.tile([128, CK], F32, tag="gcb")
    bcb = lnw.tile([128, CK], F32, tag="bcb")
    bvb = lnw.tile([128, CQ], F32, tag="bvb")
    nc.gpsimd.dma_start(out=gqb[:, :], in_=_bcast_ap(t["gamma_q"], 128, CQ))
    nc.gpsimd.dma_start(out=bqb[:, :], in_=_bcast_ap(t["beta_q"], 128, CQ))
    nc.gpsimd.dma_start(out=gcb[:, :], in_=_bcast_ap(t["gamma_ctx"], 128, CK))
    nc.gpsimd.dma_start(out=bcb[:, :], in_=_bcast_ap(t["beta_ctx"], 128, CK))
    nc.gpsimd.dma_start(out=bq_cols[:, :], in_=_cols_ap(t["bq"], KC_Q))
    nc.gpsimd.dma_start(out=bvb[:, :], in_=_bcast_ap(t["bv"], 128, CQ))

    def load_w(dram, n_chunks, tagp):
        """fp32 pair-loads -> bf16 tiles (Pool copies)."""
        tiles = []
        for p2 in range((n_chunks + 1) // 2):
            npair = min(2, n_chunks - p2 * 2)
            wf = wfp.tile([128, 2, CQ], F32, tag="wf", name=f"wf{tagp}{p2}")
            nc.scalar.dma_start(
                out=wf[:, 0:npair, :],
                in_=dram.ap()[p2 * 256:p2 * 256 + npair * 128, :].rearrange(
                    "(t p) c -> p t c", p=128))
            for j in range(npair):
                kc = p2 * 2 + j
                wb = lnw.tile([128, CQ], BF16, tag=f"w{tagp}{kc}",
                              name=f"w{tagp}{kc}")
                nc.gpsimd.tensor_copy(out=wb[:, :], in_=wf[:, j, :])
                tiles.append(wb)
        return tiles

    def ln_stats(x_dram, i0, C, n_sub, sub, mv, tagsfx=""):
        """Pair-load two [128, C] token tiles, bn stats -> mv[:, :, :],
        rsqrt(var+eps) = exp(-0.5*ln(var+eps)) on ACT (exp-table-resident).
        Returns the xf tile (caller may reuse or drop it)."""
        xf = xfp.tile([128, 2, C], F32, tag="xf", name=f"xf{i0}_{C}{tagsfx}")
        nc.scalar.dma_start(
            out=xf[:, :, :],
            in_=x_dram.ap()[i0 * 128:(i0 + 2) * 128, :].rearrange(
                "(t p) c -> p t c", p=128))
        for j in range(2):
            st = stp.tile([128, n_sub, 6], F32, tag="st",
                          name=f"st{i0 + j}_{C}{tagsfx}")
            for s in range(n_sub):
                nc.vector.bn_stats(out=st[:, s, :],
                                   in_=xf[:, j, s * sub:(s + 1) * sub])
            nc.vector.bn_aggr(out=mv[:, j, :], in_=st[:, :, :])
        nc.scalar.activation(out=mv[:, :, 1:2], in_=mv[:, :, 1:2],
                             func=AF.Ln, bias=eps_t[:, :], scale=1.0)
        nc.scalar.activation(out=mv[:, :, 1:2], in_=mv[:, :, 1:2],
                             func=AF.Exp, scale=-0.5)
        return xf

    def ln_apply(xf, i0, C, mv, gb, bb, xT, col0):
        """Apply (x-mu)*gamma*rsqrt + beta, one xbar transpose per tile."""
        for j in range(2):
            nc.vector.scalar_tensor_tensor(
                out=xf[:, j, :], in0=xf[:, j, :], scalar=mv[:, j, 0:1],
                in1=gb[:, :], op0=OP.subtract, op1=OP.mult)
            yb = bfp.tile([128, C], BF16, tag="yb", name=f"yb{i0 + j}_{C}")
            nc.vector.scalar_tensor_tensor(
                out=yb[:, :], in0=xf[:, j, :], scalar=mv[:, j, 1:2],
                in1=bb[:, :], op0=OP.mult, op1=OP.add)
            nc.sync.dma_start(
                out=xT[:, :, col0 + j * 128:col0 + (j + 1) * 128],
                in_=yb[:, :], transpose=True)

    def ln_pair(x_dram, i0, C, n_sub, sub, gb, bb, xT, col0):
        mv = stp.tile([128, 2, 2], F32, tag="mv", name=f"mv{i0}_{C}")
        xf = ln_stats(x_dram, i0, C, n_sub, sub, mv)
        ln_apply(xf, i0, C, mv, gb, bb, xT, col0)

    # ---- query side ----
    wq = load_w(t["Wq"], KC_Q, "q")
    for t2 in range(NQ2):
        xqT = xTp.tile([128, KC_Q, QC], BF16, tag="xT", name=f"xqT_{t2}")
        ln_pair(t["xq"], t2 * 4 + 0, CQ, 2, 512, gqb, bqb, xqT, 0)
        ln_pair(t["xq"], t2 * 4 + 2, CQ, 2, 512, gqb, bqb, xqT, 256)
        for oc in range(KC_Q):
            ps = pps.tile([128, QC], F32, tag="pp", name=f"psq{oc}_{t2}")
            for kc in range(KC_Q):
                nc.tensor.matmul(ps[:, :],
                                 wq[kc][:, oc * 128:(oc + 1) * 128],
                                 xqT[:, kc, :],
                                 start=(kc == 0), stop=(kc == KC_Q - 1))
            nc.vector.tensor_scalar_add(out=qTc[oc][:, t2 * QC:(t2 + 1) * QC],
                                        in0=ps[:, :],
                                        scalar1=bq_cols[:, oc:oc + 1])

    wk = load_w(t["Wk"], KC_C, "k")
    wv = load_w(t["Wv"], KC_C, "v")
    for t4 in range(NG):
        xcT = xTp.tile([128, KC_C, QC], BF16, tag="xT", name=f"xcT_{t4}")
        ln_pair(t["xc"], t4 * 4 + 0, CK, 3, 256, gcb, bcb, xcT, 0)
        ln_pair(t["xc"], t4 * 4 + 2, CK, 3, 256, gcb, bcb, xcT, 256)
        for oc in range(KC_Q):
            ps = pps.tile([128, QC], F32, tag="pp", name=f"psk{oc}_{t4}")
            for kc in range(KC_C):
                nc.tensor.matmul(ps[:, :],
                                 wk[kc][:, oc * 128:(oc + 1) * 128],
                                 xcT[:, kc, :],
                                 start=(kc == 0), stop=(kc == KC_C - 1))
            nc.vector.tensor_copy(out=kTc[oc][t4][:, :], in_=ps[:, :])
        for ki in range(4):
            kt = t4 * 4 + ki
            for v2 in range(CQ // QC):
                ps = pps.tile([128, QC], F32, tag="pp", name=f"psv{kt}_{v2}")
                for kc in range(KC_C):
                    nc.tensor.matmul(ps[:, :],
                                     xcT[:, kc, ki * 128:(ki + 1) * 128],
                                     wv[kc][:, v2 * QC:(v2 + 1) * QC],
                                     start=(kc == 0), stop=(kc == KC_C - 1))
                nc.vector.tensor_tensor(
                    out=v_g[t4][:, ki, v2 * 8:(v2 + 1) * 8, 0:D],
                    in0=ps[:, :].rearrange("p (h d) -> p h d", d=D),
                    in1=bvb[:, v2 * QC:(v2 + 1) * QC].rearrange(
                        "p (h d) -> p h d", d=D),
                    op=OP.add)
            nc.vector.memset(v_g[t4][:, ki, :, D:D + 1], 1.0)

    proj_es.close()

    # ---- Wo load + bf16 copy + bob (post-proj SBUF) ----
    wop_es = ExitStack()
    wop = wop_es.enter_context(tc.tile_pool(name="wop", bufs=1))
    wfp2 = wop_es.enter_context(tc.tile_pool(name="wfp2", bufs=2))
    wo = wop.tile([128, KC_Q, CQ], BF16, name="wo")
    bob = wop.tile([128, CQ], F32, name="bob")
    nc.gpsimd.dma_start(out=bob[:, :], in_=_bcast_ap(t["bo"], 128, CQ))
    for p2 in range(KC_Q // 2):
        wf = wfp2.tile([128, 2, CQ], F32, tag="wf", name=f"wfo{p2}")
        nc.scalar.dma_start(
            out=wf[:, :, :],
            in_=t["Wo"].ap()[p2 * 256:(p2 + 1) * 256, :].rearrange(
                "(t p) c -> p t c", p=128))
        for j in range(2):
            nc.gpsimd.tensor_copy(out=wo[:, p2 * 2 + j, :], in_=wf[:, j, :])

    # ---------------- phase 2: attention ----------------
    if "attn" not in stages:
        with tc.tile_pool(name="fl", bufs=1) as fl:
            fb = fl.tile([128, QC], F32, name="fb")
            nc.vector.tensor_copy(out=fb[:, :], in_=qTc[0][:, 0:QC])
            nc.sync.dma_start(out=out.ap()[0:128, 0:QC], in_=fb[:, :])
        wop_es.close()
        es.close()
        return

    with tc.tile_pool(name="scps", bufs=2, space="PSUM") as scps, \
         tc.tile_pool(name="attps", bufs=2, space="PSUM") as attps, \
         tc.tile_pool(name="ep", bufs=12) as ep, \
         tc.tile_pool(name="rp", bufs=2) as rp, \
         tc.tile_pool(name="tmp1", bufs=2) as tmp1p:

        for hp in range(H // 2):
            att = {}
            for par in range(2):
                att[par] = attps.tile([D + 1, NQ], F32, tag="att",
                                      name=f"attp{2 * hp + par}")

            def emit_att(kt):
                for par in range(2):
                    h = 2 * hp + par
                    for q2 in range(NQ2):
                        nc.tensor.matmul(
                            att[par][:, q2 * QC:(q2 + 1) * QC],
                            v_g[kt // 4][:, kt % 4, h, :],
                            es_e[kt][par][:, q2 * QC:(q2 + 1) * QC],
                            start=(kt == 0), stop=(kt == NKT - 1))

            es_e = {}
            for kt in range(NKT):
                sc = {}
                for par in range(2):
                    sc[par] = scps.tile([128, NQ], F32, tag="sc",
                                        name=f"sc{2 * hp + par}_{kt}")
                for q2 in range(NQ2):
                    for par in range(2):
                        lo = par * 64
                        nc.tensor.matmul(
                            sc[par][:, q2 * QC:(q2 + 1) * QC],
                            kTc[hp][kt // 4][lo:lo + 64,
                                             (kt % 4) * 128:(kt % 4 + 1) * 128],
                            qTc[hp][lo:lo + 64, q2 * QC:(q2 + 1) * QC],
                            start=True, stop=True)
                es_e[kt] = {}
                for par in range(2):
                    e = ep.tile([128, NQ], BF16, tag="e",
                                name=f"e{2 * hp + par}_{kt}")
                    nc.scalar.activation(out=e[:, :], in_=sc[par][:, :],
                                         func=AF.Exp, scale=SM_SCALE)
                    es_e[kt][par] = e
                if kt >= 1:
                    emit_att(kt - 1)
                    del es_e[kt - 1]
            emit_att(NKT - 1)

            for par in range(2):
                h = 2 * hp + par
                rec = rp.tile([1, NQ], F32, tag="rec", name=f"rec{h}")
                nc.vector.reciprocal(out=rec[:, :], in_=att[par][D:D + 1, :])
                sd = scr.tile([1, NQ], F32, tag="sd", name=f"sd{h}")
                nc.sync.dma_start(out=sd[:, :], in_=rec[:, :])
                rb = rp.tile([64, NQ], F32, tag="rb", name=f"rb{h}")
                nc.sync.dma_start(
                    out=rb[:, :],
                    in_=bass.AP(tensor=sd.tensor, offset=sd.offset,
                                ap=[[0, 64], [1, NQ]]))
                if par == 0:
                    nc.vector.tensor_tensor(out=attT[0:64, hp, :],
                                            in0=att[par][0:D, :],
                                            in1=rb[:, :], op=OP.mult)
                else:
                    tm = tmp1p.tile([64, NQ], BF16, tag="tm", name=f"tm{h}")
                    nc.vector.tensor_tensor(out=tm[:, :],
                                            in0=att[par][0:D, :],
                                            in1=rb[:, :], op=OP.mult)
                    nc.sync.dma_start(out=attT[64:128, hp, :], in_=tm[:, :])

    # ---------------- phase 3: out projection ----------------
    with tc.tile_pool(name="ops", bufs=2, space="PSUM") as ops, \
         tc.tile_pool(name="op", bufs=2) as op_pool:
        if "out" not in stages:
            fb2 = op_pool.tile([128, QC], F32, name="fb2")
            nc.vector.tensor_copy(out=fb2[:, :], in_=attT[:, 0, 0:QC])
            nc.sync.dma_start(out=out.ap()[0:128, 0:QC], in_=fb2[:, :])
        for qt in range(NQT if "out" in stages else 0):
            osb = op_pool.tile([128, CQ], F32, tag="osb", name=f"osb{qt}")
            for cc in range(CQ // QC):
                ps = ops.tile([128, QC], F32, tag="opp", name=f"pso{qt}_{cc}")
                for kc in range(KC_Q):
                    nc.tensor.matmul(
                        ps[:, :],
                        attT[:, kc, qt * 128:(qt + 1) * 128],
                        wo[:, kc, cc * QC:(cc + 1) * QC],
                        start=(kc == 0), stop=(kc == KC_Q - 1))
                nc.vector.tensor_tensor(out=osb[:, cc * QC:(cc + 1) * QC],
                                        in0=ps[:, :],
                                        in1=bob[:, cc * QC:(cc + 1) * QC],
                                        op=OP.add)
            nc.sync.dma_start(out=out.ap()[qt * 128:(qt + 1) * 128, :],
                              in_=osb[:, :])

    wop_es.close()
    es.close()


def build():
    nc = bass.Bass("TRN2", target_bir_lowering=False, debug=False,
                   num_devices=N_CORES)
    t = {
        "xq": nc.dram_tensor("xq", [NQ, CQ], F32, kind="ExternalInput"),
        "xc": nc.dram_tensor("xc", [NK, CK], F32, kind="ExternalInput"),
        "Wq": nc.dram_tensor("Wq", [CQ, CQ], F32, kind="ExternalInput"),
        "Wk": nc.dram_tensor("Wk", [CK, CQ], F32, kind="ExternalInput"),
        "Wv": nc.dram_tensor("Wv", [CK, CQ], F32, kind="ExternalInput"),
        "Wo": nc.dram_tensor("Wo", [CQ, CQ], F32, kind="ExternalInput"),
        "bq": nc.dram_tensor("bq", [CQ], F32, kind="ExternalInput"),
        "bk": nc.dram_tensor("bk", [CQ], F32, kind="ExternalInput"),
        "bv": nc.dram_tensor("bv", [CQ], F32, kind="ExternalInput"),
        "bo": nc.dram_tensor("bo", [CQ], F32, kind="ExternalInput"),
        "gamma_q": nc.dram_tensor("gamma_q", [CQ], F32, kind="ExternalInput"),
        "beta_q": nc.dram_tensor("beta_q", [CQ], F32, kind="ExternalInput"),
        "gamma_ctx": nc.dram_tensor("gamma_ctx", [CK], F32, kind="ExternalInput"),
        "beta_ctx": nc.dram_tensor("beta_ctx", [CK], F32, kind="ExternalInput"),
    }
    out = nc.dram_tensor("out", [NQ, CQ], F32, kind="ExternalOutput")
    with tile.TileContext(nc) as tc:
        _emit(tc, t, out)
    _split_excess_waits(nc)
    return nc


_NC = None


def _in_maps(inputs):
    q = np.ascontiguousarray(np.asarray(inputs["query_tokens"], dtype=np.float32))
    c = np.ascontiguousarray(np.asarray(inputs["context_tokens"], dtype=np.float32))
    shared = {k: np.ascontiguousarray(np.asarray(inputs[k], dtype=np.float32))
              for k in ("Wq", "Wk", "Wv", "Wo", "bq", "bk", "bv", "bo",
                        "gamma_q", "beta_q", "gamma_ctx", "beta_ctx")}
    maps = []
    for core in range(N_CORES):
        b, half = core // 2, core % 2
        m = dict(shared)
        m["xq"] = np.ascontiguousarray(q[b, half * NQ:(half + 1) * NQ, :])
        m["xc"] = np.ascontiguousarray(c[b])
        maps.append(m)
    return maps


def run_sharded(inputs, **kwargs):
    global _NC
    if _NC is None:
        _NC = build()
    return run_bass_kernel_spmd(_NC, _in_maps(inputs),
                                core_ids=list(range(N_CORES)), **kwargs)


def kernel(**inputs) -> np.ndarray:
    res = run_sharded(inputs)
    out = np.empty((B, NQ_FULL, CQ), np.float32)
    for core in range(N_CORES):
        b, half = core // 2, core % 2
        out[b, half * NQ:(half + 1) * NQ, :] = res.results[core]["out"]
    return out
